# revision 1
# baseline (speedup 1.0000x reference)
"""Causal self-attention (GQA + RoPE) Trainium2 Bass kernel, 8-way sharded.

Sharding: core c -> batch b = c // 2, head-half hh = c % 2.
Each core computes the qkv projection, attention and output projection for
its batch and its 16 query heads / 4 kv heads (kv groups kept whole); the
output projection is a row-shard of Wproj, so the two cores of a batch
produce partial sums that the host adds.

Device-side layout tricks (host prepares):
  - x is fed pre-transposed (xT [C, T]) so the qkv matmul needs no on-device
    transpose of x.
  - Wq/Wk columns are de-interleaved per head (even rope pairs then odd), so
    RoPE becomes the rotate-half form with free-dim slices only.
  - scores are computed transposed (scoresT = k_tile^T-matmul) so the
    attention-weights matmul needs no transposes; softmax normalization is
    reconstructed via a ones-row matmul + reciprocal + PE broadcast.

All PE matmuls run in float32r (full rate for moving dim >= 256).
"""

import os

os.environ.setdefault("JAX_PLATFORMS", "axon")

import numpy as np

B, T, C = 4, 1024, 4096
H, KV, HD = 32, 8, 128
REP = H // KV  # 4

NQ = 16      # q heads per core
NKV = 4      # kv heads per core
QK_HEADS = NQ + NKV          # 20 rope'd/transposed heads
COLS = (NQ + 2 * NKV) * HD   # 3072 local qkv cols: q0..15 k0..3 v0..3
NTT = T // 128               # 8 token tiles
SCALE = float(1.0 / np.sqrt(np.float32(HD)).astype(np.float32))

_CACHE: dict = {}


def _build_nc():
    import concourse.mybir as mybir
    import concourse.tile as tile
    from concourse import bacc
    from concourse.bass import ts
    from concourse.masks import make_identity

    f32 = mybir.dt.float32
    f32r = mybir.dt.float32r
    Exp = mybir.ActivationFunctionType.Exp

    nc = bacc.Bacc(None, target_bir_lowering=False, debug=False)

    xT_d = nc.dram_tensor("xT", [C, T], f32r, kind="ExternalInput")
    # [colhalf, chalf, j(256-col chunk), cc(128-row chunk), 128, 256]
    wqkv_d = nc.dram_tensor("wqkv", [2, 2, 6, 16, 128, 256], f32r, kind="ExternalInput")
    # [ccol(512-col chunk), ycc(128-row chunk), 128, 512]
    wproj_d = nc.dram_tensor("wproj", [8, 16, 128, 512], f32r, kind="ExternalInput")
    cos_d = nc.dram_tensor("cosn", [T, 64], f32, kind="ExternalInput")
    sin_d = nc.dram_tensor("sinn", [T, 64], f32, kind="ExternalInput")
    # mask_rel[d][p][f] = 1.0 if 128*d + p <= f else 0.0
    masks_d = nc.dram_tensor("masks", [4, 128, 512], f32, kind="ExternalInput")
    out_d = nc.dram_tensor("out", [T, C], f32, kind="ExternalOutput")
    # scratch: q/k transposed [head, hd=128, T]; v natural [T, 512]
    qkT_d = nc.dram_tensor("qkT_scratch", [QK_HEADS, 128, T], f32r)
    v_d = nc.dram_tensor("v_scratch", [T, NKV * HD], f32r)

    with (
        tile.TileContext(nc) as tc,
        tc.tile_pool(name="const", bufs=1) as const_p,
        tc.tile_pool(name="psA", bufs=3, space="PSUM") as psA,
    ):
        ident = const_p.tile([128, 128], f32)
        make_identity(nc, ident[:])
        ones0 = const_p.tile([128, 1], f32)
        nc.vector.memset(ones0[:], 1.0)
        ones_red = const_p.tile([128, 1], f32r)
        nc.scalar.copy(out=ones_red[:], in_=ones0[:])
        ones_row0 = const_p.tile([1, 128], f32)
        nc.vector.memset(ones_row0[:], 1.0)
        ones_row = const_p.tile([1, 128], f32r)
        nc.scalar.copy(out=ones_row[:], in_=ones_row0[:])
        cos_sb = const_p.tile([128, NTT, 64], f32)
        sin_sb = const_p.tile([128, NTT, 64], f32)

        # ================= PHASE 1: qkv = x @ Wqkv (+RoPE, +transposes) =====
        # W is streamed exactly once; x is re-streamed per column half; the
        # C-dim is split in two halves accumulated through SBUF (acc).
        xT_r = xT_d.rearrange("(cc p) t -> p cc t", p=128)  # [128, 32, 1024]
        with (
            tc.tile_pool(name="x", bufs=6) as x_p,
            tc.tile_pool(name="w", bufs=2) as w_p,
            tc.tile_pool(name="acc", bufs=1) as acc_p,
            tc.tile_pool(name="rope", bufs=3) as rope_p,
            tc.tile_pool(name="rtmp", bufs=3) as rtmp_p,
            tc.tile_pool(name="tstage", bufs=3) as tstage_p,
            tc.tile_pool(name="vstage", bufs=2) as vstage_p,
            tc.tile_pool(name="psT", bufs=3, space="PSUM") as psT,
        ):
            # second colhalf processes chalves in reverse so its first
            # segment reuses the x tiles already resident in SBUF
            segs = [(0, 0), (0, 1), (1, 1), (1, 0)]
            seg_tiles: dict = {}

            def emit_x(si_, q_, split=False):
                ch_, cf_ = segs[si_]
                xt = x_p.tile(
                    [128, 16, 256], f32r, tag="x", name=f"x{ch_}{cf_}{q_}"
                )
                base = 16 * cf_
                if split:
                    nc.sync.dma_start(
                        out=xt[:, 0:8, :], in_=xT_r[:, base : base + 8, ts(q_, 256)]
                    )
                    nc.sync.dma_start(
                        out=xt[:, 8:16, :],
                        in_=xT_r[:, base + 8 : base + 16, ts(q_, 256)],
                    )
                else:
                    nc.sync.dma_start(
                        out=xt[:], in_=xT_r[:, base : base + 16, ts(q_, 256)]
                    )
                seg_tiles.setdefault(si_, {})[q_] = xt

            emit_x(0, 0)
            acc = None
            pending = []
            for si_seg, (colhalf, chalf) in enumerate(segs):
                first = si_seg % 2 == 0  # first segment of this colhalf
                if first:
                    acc = acc_p.tile(
                        [128, NTT, 1536], f32, tag="acc", name=f"acc{colhalf}"
                    )
                if si_seg == 2:
                    seg_tiles[2] = seg_tiles[1]  # (1,1) reuses (0,1)'s x tiles
                xq = seg_tiles[si_seg]
                for j in range(6):
                    wt = w_p.tile([128, 16, 256], f32r, tag="w")
                    nc.sync.dma_start(
                        out=wt[:],
                        in_=wqkv_d[colhalf, chalf, j].rearrange("cc p f -> p cc f"),
                    )
                    if j == 0:
                        if si_seg == 0:
                            emit_x(0, 1)
                        if si_seg != 2:
                            emit_x(si_seg, 2)
                            emit_x(si_seg, 3)
                        if si_seg == 0:
                            nc.sync.dma_start(
                                out=cos_sb[:],
                                in_=cos_d.rearrange("(tt p) j -> p tt j", p=128),
                            )
                            nc.sync.dma_start(
                                out=sin_sb[:],
                                in_=sin_d.rearrange("(tt p) j -> p tt j", p=128),
                            )
                    elif j == 2 and si_seg + 1 < len(segs) and si_seg + 1 != 2:
                        emit_x(si_seg + 1, 0)
                        emit_x(si_seg + 1, 1)
                    for tt in range(NTT):
                        ps = psA.tile([128, 256], f32, tag="psA")
                        for cc in range(16):
                            nc.tensor.matmul(
                                ps[:],
                                xq[tt // 2][:, cc, ts(tt % 2, 128)],
                                wt[:, cc, :],
                                start=(cc == 0),
                                stop=(cc == 15),
                            )
                        dst = acc[:, tt, ts(j, 256)]
                        if first:
                            nc.scalar.copy(out=dst, in_=ps[:])
                            continue
                        nc.vector.tensor_add(dst, ps[:], dst)
                        if tt % 2 == 0:
                            continue
                        # (tt-1, tt) finalized -> rope batch + spill; the
                        # transposes of the previous batch are emitted now
                        # (one-batch software pipeline) so PE never waits on
                        # the rope chain.
                        slot = colhalf * 6 + j  # global 256-col chunk
                        t2p = tt - 1
                        if slot < 10:
                            a = acc[:, t2p : tt + 1, ts(j, 256)].rearrange(
                                "p t (h x j) -> p t h x j", x=2, j=64
                            )
                            cosb = (
                                cos_sb[:, t2p : tt + 1, :]
                                .unsqueeze(2)
                                .broadcast_to([128, 2, 2, 64])
                            )
                            sinb = (
                                sin_sb[:, t2p : tt + 1, :]
                                .unsqueeze(2)
                                .broadcast_to([128, 2, 2, 64])
                            )
                            rt = rope_p.tile([128, 2, 2, 2, 64], f32, tag="rt")
                            t0 = rtmp_p.tile([128, 2, 2, 64], f32, tag="t0")
                            t1 = rtmp_p.tile([128, 2, 2, 64], f32, tag="t1")
                            t2 = rtmp_p.tile([128, 2, 2, 64], f32, tag="t2")
                            t3 = rtmp_p.tile([128, 2, 2, 64], f32, tag="t3")
                            nc.gpsimd.tensor_mul(t0[:], a[:, :, :, 0, :], cosb)
                            nc.gpsimd.tensor_mul(t1[:], a[:, :, :, 1, :], sinb)
                            nc.vector.tensor_sub(rt[:, :, :, 0, :], t0[:], t1[:])
                            nc.gpsimd.tensor_mul(t2[:], a[:, :, :, 1, :], cosb)
                            nc.gpsimd.tensor_mul(t3[:], a[:, :, :, 0, :], sinb)
                            nc.vector.tensor_add(rt[:, :, :, 1, :], t2[:], t3[:])
                            for ppend in pending:
                                ppend()
                            pending = []

                            def mk(rt_, slot_, t2p_):
                                def emit():
                                    for ttl in range(2):
                                        for hh in range(2):
                                            h = 2 * slot_ + hh
                                            pt = psT.tile([128, 128], f32, tag="psT")
                                            nc.tensor.transpose(
                                                pt[:],
                                                rt_[:, ttl, hh].rearrange(
                                                    "p x j -> p (x j)"
                                                ),
                                                ident[:],
                                            )
                                            st = tstage_p.tile(
                                                [128, 128], f32r, tag="ts"
                                            )
                                            nc.scalar.copy(out=st[:], in_=pt[:])
                                            nc.sync.dma_start(
                                                out=qkT_d[h, :, ts(t2p_ + ttl, 128)],
                                                in_=st[:],
                                            )
                                return emit

                            pending.append(mk(rt, slot, t2p))
                        else:
                            vs = vstage_p.tile([128, 2, 256], f32r, tag="vs")
                            nc.scalar.copy(
                                out=vs[:], in_=acc[:, t2p : tt + 1, ts(j, 256)]
                            )
                            nc.sync.dma_start(
                                out=v_d[
                                    128 * t2p : 128 * (tt + 1), ts(slot - 10, 256)
                                ].rearrange("(t p) f -> p t f", p=128),
                                in_=vs[:],
                            )
            for ppend in pending:
                ppend()
            pending = []

        # ================= PHASE 2: attention ==============================
        with (
            tc.tile_pool(name="yt", bufs=NQ) as yt_p,
            tc.tile_pool(name="vsb", bufs=1) as vsb_p,
            tc.tile_pool(name="msk", bufs=1) as msk_p,
            tc.tile_pool(name="wp", bufs=2) as wp_p,
            tc.tile_pool(name="ostage", bufs=2) as ostage_p,
        ):
            yts = [yt_p.tile([128, T], f32r, tag="yt", name=f"yt{i}") for i in range(NQ)]
            v_sb = vsb_p.tile([128, NTT, NKV * HD], f32r)
            mask_sb = msk_p.tile([128, 4, 512], f32)
            wps = {}

            with (
                tc.tile_pool(name="qt", bufs=2) as qt_p,
                tc.tile_pool(name="kt", bufs=2) as kt_p,
                tc.tile_pool(name="exp", bufs=4) as exp_p,
                tc.tile_pool(name="small", bufs=3) as small_p,
                tc.tile_pool(name="wp", bufs=2) as wp_p,
                tc.tile_pool(name="ostage", bufs=3) as ostage_p,
                tc.tile_pool(name="psY", bufs=3, space="PSUM") as psY,
                tc.tile_pool(name="psS", bufs=2, space="PSUM") as psS,
            ):
                for g in range(NKV):
                    kt = kt_p.tile([128, T], f32r, tag="kt")
                    nc.sync.dma_start(out=kt[:], in_=qkT_d[NQ + g])
                    if g >= 1:
                        # prefetch the first Wproj block during attention,
                        # quartered to avoid head-of-line blocking qt loads
                        if g == 1:
                            wps[0] = wp_p.tile(
                                [128, 16, 512], f32r, tag="wp", name="wp0"
                            )
                        for qq in ([0, 1] if g == 1 else [2] if g == 2 else [3]):
                            nc.sync.dma_start(
                                out=wps[0][:, 4 * qq : 4 * (qq + 1), :],
                                in_=wproj_d[0, 4 * qq : 4 * (qq + 1)].rearrange(
                                    "y p f -> p y f"
                                ),
                            )
                    for r in range(REP):
                        hq = g * REP + r
                        qt = qt_p.tile([128, T], f32r, tag="qt")
                        nc.sync.dma_start(out=qt[:], in_=qkT_d[hq])
                        if g == 0 and r == 0:
                            # behind the first kt/qt so those win the queue
                            nc.sync.dma_start(
                                out=mask_sb[:],
                                in_=masks_d.rearrange("d p f -> p d f"),
                            )
                            vr = v_d.rearrange("(tt p) f -> p tt f", p=128)
                            nc.sync.dma_start(out=v_sb[:, 0:4, :], in_=vr[:, 0:4, :])
                            nc.sync.dma_start(out=v_sb[:, 4:8, :], in_=vr[:, 4:8, :])
                        for chunk in range(4):
                            tq0 = 256 * chunk
                            ns = 2 * (chunk + 1)
                            py = psY.tile([128, 256], f32, tag="psY")
                            psum = psS.tile([1, 256], f32, tag="psS")
                            for pair in range(ns // 2):
                                si0 = 2 * pair
                                pss = psA.tile([128, 2, 256], f32, tag="psA")
                                for i in range(2):
                                    nc.tensor.matmul(
                                        pss[:, i, :],
                                        kt[:, ts(si0 + i, 128)],
                                        qt[:, tq0 : tq0 + 256],
                                        start=True,
                                        stop=True,
                                    )
                                et = exp_p.tile([128, 2, 256], f32r, tag="exp")
                                nc.scalar.activation(
                                    out=et[:], in_=pss[:], func=Exp, scale=SCALE
                                )
                                if si0 == 2 * chunk:  # diagonal pair
                                    nc.vector.tensor_mul(
                                        et[:], et[:], mask_sb[:, 0:2, 0:256]
                                    )
                                for i in range(2):
                                    si = si0 + i
                                    nc.tensor.matmul(
                                        py[:],
                                        v_sb[:, si, ts(g, 128)],
                                        et[:, i, :],
                                        start=(si == 0),
                                        stop=(si == ns - 1),
                                    )
                                    nc.tensor.matmul(
                                        psum[:],
                                        ones_red[:],
                                        et[:, i, :],
                                        start=(si == 0),
                                        stop=(si == ns - 1),
                                    )
                            recip = small_p.tile([1, 256], f32r, tag="recip")
                            with nc.allow_low_precision(reason="fp32r softmax recip"):
                                nc.vector.reciprocal(out=recip[:], in_=psum[:])
                            prb = psS.tile([128, 256], f32, tag="psS")
                            nc.tensor.matmul(
                                prb[:],
                                ones_row[:],
                                recip[:],
                                start=True,
                                stop=True,
                            )
                            rb = small_p.tile([128, 256], f32, tag="rb")
                            nc.scalar.copy(out=rb[:], in_=prb[:])
                            nc.vector.tensor_mul(
                                yts[hq][:, tq0 : tq0 + 256], py[:], rb[:]
                            )

                # ============= PHASE 3: out = y @ Wproj (row shard) ============
                for ccol in range(8):
                    if ccol in wps:
                        wp = wps[ccol]
                    else:
                        wp = wp_p.tile([128, 16, 512], f32r, tag="wp")
                        nc.sync.dma_start(
                            out=wp[:, 0:8, :],
                            in_=wproj_d[ccol, 0:8].rearrange("y p f -> p y f"),
                        )
                        nc.sync.dma_start(
                            out=wp[:, 8:16, :],
                            in_=wproj_d[ccol, 8:16].rearrange("y p f -> p y f"),
                        )
                    for tt in range(NTT):
                        po = psA.tile([128, 512], f32, tag="psA")
                        for ycc in range(16):
                            nc.tensor.matmul(
                                po[:],
                                yts[ycc][:, ts(tt, 128)],
                                wp[:, ycc, :],
                                start=(ycc == 0),
                                stop=(ycc == 15),
                            )
                        ot = ostage_p.tile([128, 512], f32, tag="os")
                        nc.scalar.copy(out=ot[:], in_=po[:])
                        nc.sync.dma_start(
                            out=out_d[ts(tt, 128), ts(ccol, 512)], in_=ot[:]
                        )

    nc.compile()
    return nc


def prep_inputs(x, Wqkv, Wproj, freqs_cos, freqs_sin):
    """Build the 8 per-core input maps (host-side shard + layout prep)."""
    x = np.asarray(x, np.float32)
    Wqkv = np.asarray(Wqkv, np.float32)
    Wproj = np.asarray(Wproj, np.float32)
    cos = np.ascontiguousarray(np.asarray(freqs_cos, np.float32))
    sin = np.ascontiguousarray(np.asarray(freqs_sin, np.float32))

    perm = np.concatenate([np.arange(0, HD, 2), np.arange(1, HD, 2)])
    masks = (
        (128 * np.arange(4)[:, None, None] + np.arange(128)[None, :, None])
        <= np.arange(512)[None, None, :]
    ).astype(np.float32)
    masks = np.ascontiguousarray(masks)

    in_maps = []
    for c in range(8):
        b, hh = divmod(c, 2)
        qcols = (hh * NQ * HD + (np.arange(NQ) * HD)[:, None] + perm[None, :]).ravel()
        kcols = (
            H * HD + hh * NKV * HD + (np.arange(NKV) * HD)[:, None] + perm[None, :]
        ).ravel()
        vcols = (
            (H + KV) * HD
            + hh * NKV * HD
            + (np.arange(NKV) * HD)[:, None]
            + np.arange(HD)[None, :]
        ).ravel()
        col_idx = np.concatenate([qcols, kcols, vcols])
        Wc = Wqkv[:, col_idx]  # [4096, 3072]
        wq = np.ascontiguousarray(
            Wc.reshape(2, 16, 128, 2, 6, 256).transpose(3, 0, 4, 1, 2, 5)
        )
        Wp = Wproj[hh * NQ * HD : (hh + 1) * NQ * HD, :]  # [2048, 4096]
        wp = np.ascontiguousarray(Wp.reshape(16, 128, 8, 512).transpose(2, 0, 1, 3))
        xT = np.ascontiguousarray(x[b].T)  # [4096, 1024]
        in_maps.append(
            {"xT": xT, "wqkv": wq, "wproj": wp, "cosn": cos, "sinn": sin,
             "masks": masks}
        )
    return in_maps


def _get_nc():
    if "nc" not in _CACHE:
        _CACHE["nc"] = _build_nc()
    return _CACHE["nc"]


def kernel(x, Wqkv, Wproj, freqs_cos, freqs_sin, mask=None):
    from concourse.bass_utils import run_bass_kernel_spmd

    nc = _get_nc()
    in_maps = prep_inputs(x, Wqkv, Wproj, freqs_cos, freqs_sin)
    res = run_bass_kernel_spmd(nc, in_maps, core_ids=list(range(8)))
    outs = [res.results[c]["out"] for c in range(8)]
    y = np.stack([outs[2 * b] + outs[2 * b + 1] for b in range(B)], axis=0)
    return y.astype(np.float32)



# revision 12
# speedup vs baseline: 1.8519x; 1.8519x over previous
"""Causal self-attention (GQA + RoPE) Trainium2 Bass kernel, 8-way sharded.

Sharding: core c -> batch b = c // 2, head-half hh = c % 2.
Each core computes qkv projection, attention and output projection for its
batch and its 16 query heads / 4 kv heads; the output projection is a
row-shard of Wproj, so the two cores of a batch produce partial sums that
the host adds.

v2 design (all-SBUF, bf16):
  - q/k projection runs W-stationary (moving operand = xT), so the PE emits
    q^T/k^T [hd, T] directly -- no PE transposes, no DRAM round trip. RoPE
    (rotate-half form via host-side column de-interleave) is applied on the
    PSUM->SBUF path by ACT (half swap) + DVE/Pool (muls) + DVE (add).
  - v runs x-stationary into natural [T, 4*HD] layout.
  - attention uses transposed scores (scoresT = kT_slice^T @ qT) so no
    transposes are needed anywhere; softmax normalization: exp tiles are
    pair-tree-summed on DVE, then ONE ones-matrix matmul per 256-col chunk
    produces the denominator already broadcast across partitions.
  - attention for kv-group g is interleaved into the qkv slots of group g+1
    so the exp (ACT) work hides under GEMM PE cycles.
  - everything bf16 except PSUM accumulation (fp32) and the final output.
"""

import os

os.environ.setdefault("JAX_PLATFORMS", "axon")

import numpy as np
import ml_dtypes

BF16 = ml_dtypes.bfloat16

B, T, C = 4, 1024, 4096
H, KV, HD = 32, 8, 128
REP = H // KV  # 4

NQ = 16      # q heads per core
NKV = 4      # kv heads per core
QK_HEADS = NQ + NKV   # 20 projected+rope'd heads per core
NCC = C // 128        # 32 contraction tiles
NTT = T // 128        # 8 token tiles
SCALE = float(1.0 / np.sqrt(np.float32(HD)).astype(np.float32))

_CACHE: dict = {}


def _build_nc():
    import concourse.mybir as mybir
    import concourse.tile as tile
    from concourse import bacc
    from concourse.bass import ts

    f32 = mybir.dt.float32
    bf16 = mybir.dt.bfloat16
    Exp = mybir.ActivationFunctionType.Exp

    nc = bacc.Bacc(None, target_bir_lowering=False, debug=False)

    xT_d = nc.dram_tensor("xT", [C, T], bf16, kind="ExternalInput")
    # [h, p(c within tile), cc, col]
    wqk_d = nc.dram_tensor("wqk", [QK_HEADS, 128, NCC, 128], bf16,
                           kind="ExternalInput")
    # [p, cc, vcol]
    wv_d = nc.dram_tensor("wv", [128, NCC, NKV * HD], bf16, kind="ExternalInput")
    # [ccol, p(y within tile), ycc, f]
    wp_d = nc.dram_tensor("wp", [8, 128, 16, 512], bf16, kind="ExternalInput")
    csg_d = nc.dram_tensor("csg", [128, T], bf16, kind="ExternalInput")
    ssg_d = nc.dram_tensor("ssg", [128, T], bf16, kind="ExternalInput")
    mask_d = nc.dram_tensor("maskd", [128, 2, 256], bf16, kind="ExternalInput")
    out_d = nc.dram_tensor("out", [T, C], f32, kind="ExternalOutput")

    with (
        tile.TileContext(nc) as tc,
        tc.tile_pool(name="const", bufs=1) as const_p,
        tc.tile_pool(name="qkT", bufs=10) as qkT_p,
        tc.tile_pool(name="yts", bufs=NQ) as yts_p,
        tc.tile_pool(name="vsb", bufs=1) as vsb_p,
        tc.tile_pool(name="sw", bufs=2) as sw_p,
        tc.tile_pool(name="m1", bufs=2) as m1_p,
        tc.tile_pool(name="m2", bufs=2) as m2_p,
        tc.tile_pool(name="et", bufs=12) as et_p,
        tc.tile_pool(name="se", bufs=5) as se_p,
        tc.tile_pool(name="t2", bufs=2) as t2_p,
        tc.tile_pool(name="rb", bufs=2) as rb_p,
        tc.tile_pool(name="psQK", bufs=2, space="PSUM") as psQK,
        tc.tile_pool(name="psS", bufs=3, space="PSUM") as psS,
        tc.tile_pool(name="psY", bufs=1, space="PSUM") as psY,
    ):
        ones128 = const_p.tile([128, 128], bf16)
        nc.vector.memset(ones128[:], 1.0)
        csg = const_p.tile([128, T], bf16)
        ssg = const_p.tile([128, T], bf16)
        mask_sb = const_p.tile([128, 2, 256], bf16)

        v_sb = vsb_p.tile([128, NTT, NKV * HD], bf16)
        yts = [yts_p.tile([128, T], bf16, tag="yt", name=f"yt{i}")
               for i in range(NQ)]
        qkT: dict = {}

        # ---- attention emission helpers (interleaved into qkv slots) ----
        et_tiles: dict = {}
        se_tiles: dict = {}

        def scores_steps(g, r):
            """Emission-step closures: scoresT + exp + mask + pair-tree sums
            for q head (g, r). One step per si pair (10 total)."""
            hq = 4 * g + r

            def mk(c, pair):
                def step():
                    qt = qkT[hq]
                    kt = qkT[NQ + g]
                    np_ = c + 1
                    pss = psS.tile([128, 2, 256], f32, tag="psS")
                    for i in range(2):
                        si = 2 * pair + i
                        nc.tensor.matmul(
                            pss[:, i, :],
                            kt[:, ts(si, 128)],
                            qt[:, ts(c, 256)],
                            start=True,
                            stop=True,
                        )
                    et = et_p.tile([128, 2, 256], bf16, tag="et")
                    nc.scalar.activation(
                        out=et[:], in_=pss[:], func=Exp, scale=SCALE,
                    )
                    pairs = et_tiles.setdefault((hq, c), [])
                    pairs.append(et)
                    if pair == c:  # diagonal pair: mask, then finish the sum
                        nc.vector.tensor_mul(et[:], et[:], mask_sb[:])
                        se = se_p.tile([128, 256], bf16, tag="se")
                        if np_ == 1:
                            nc.vector.tensor_add(se[:], et[:, 0, :], et[:, 1, :])
                        else:
                            t2 = t2_p.tile([128, 2, 256], bf16, tag="t2")
                            nc.vector.tensor_add(t2[:], pairs[0][:], pairs[1][:])
                            for k in range(2, np_):
                                nc.vector.tensor_add(t2[:], t2[:], pairs[k][:])
                            nc.vector.tensor_add(se[:], t2[:, 0, :], t2[:, 1, :])
                        se_tiles[(hq, c)] = se
                return step

            return [mk(c, pair) for c in range(4) for pair in range(c + 1)]

        def avdn_steps(g, r):
            """Emission-step closures: AV + broadcast denominator + normalize
            for q head (g, r). One step per chunk plus one per chunk-pair."""
            hq = 4 * g + r
            state: dict = {}

            def mk_av(c):
                def step():
                    c2, i = divmod(c, 2)
                    if i == 0:
                        state["py"] = psY.tile([128, 2, 256], f32, tag="psY", name="py")
                        state["dn"] = psS.tile([128, 2, 256], f32, tag="psS", name="dn")
                    py, dn = state["py"], state["dn"]
                    ns = 2 * (c + 1)
                    pairs = et_tiles.pop((hq, c))
                    for si in range(ns):
                        nc.tensor.matmul(
                            py[:, i, :],
                            v_sb[:, si, ts(g, 128)],
                            pairs[si // 2][:, si % 2, :],
                            start=(si == 0),
                            stop=(si == ns - 1),
                        )
                    se = se_tiles.pop((hq, c))
                    nc.tensor.matmul(
                        dn[:, i, :], ones128[:], se[:], start=True, stop=True
                    )
                return step

            def mk_norm(c2):
                def step():
                    py, dn = state["py"], state["dn"]
                    rb = rb_p.tile([128, 2, 256], f32, tag="rb")
                    nc.vector.reciprocal(out=rb[:], in_=dn[:])
                    nc.vector.tensor_mul(
                        yts[hq][:, ts(c2, 512)].rearrange(
                            "p (a b) -> p a b", a=2),
                        py[:], rb[:],
                    )
                return step

            out = []
            for c2 in range(2):
                out.append(mk_av(2 * c2))
                out.append(mk_av(2 * c2 + 1))
                out.append(mk_norm(c2))
            return out

        def interleave(primary, inserts):
            """Emit primary closures with inserts spread evenly between."""
            n, m = len(primary), len(inserts)
            ii = 0
            for k, p in enumerate(primary):
                p()
                want = (k + 1) * m // n
                while ii < want:
                    inserts[ii]()
                    ii += 1
            while ii < m:
                inserts[ii]()
                ii += 1

        # ================= PHASE A+B: qkv + interleaved attention =========
        with (
            tc.tile_pool(name="x", bufs=1) as x_p,
            tc.tile_pool(name="wv", bufs=4) as wv_p,
            tc.tile_pool(name="wqk", bufs=3) as wqk_p,
        ):
            xT_sb = x_p.tile([128, NCC, T], bf16)
            xr = xT_d.rearrange("(cc p) t -> p cc t", p=128)
            for ch in range(8):
                nc.sync.dma_start(
                    out=xT_sb[:, ts(ch, 4), :], in_=xr[:, ts(ch, 4), :]
                )
            nc.sync.dma_start(out=csg[:], in_=csg_d[:])
            nc.sync.dma_start(out=ssg[:], in_=ssg_d[:])
            nc.sync.dma_start(out=mask_sb[:], in_=mask_d[:])

            # ---- v: x-stationary, natural [T, 512] ----
            wv_t = {}
            for ch in range(4):
                wv_t[ch] = wv_p.tile([128, 8, NKV * HD], bf16, tag="wv",
                                     name=f"wv{ch}")
                nc.sync.dma_start(out=wv_t[ch][:], in_=wv_d[:, ts(ch, 8), :])
            for tt in range(NTT):
                psv = psQK.tile([128, 512], f32, tag="psQK")
                for cc in range(NCC):
                    nc.tensor.matmul(
                        psv[:],
                        xT_sb[:, cc, ts(tt, 128)],
                        wv_t[cc // 8][:, cc % 8, :],
                        start=(cc == 0),
                        stop=(cc == NCC - 1),
                    )
                nc.scalar.copy(out=v_sb[:, tt, :], in_=psv[:])

            # ---- q/k heads in groups, attention lagging one group ----
            # head order per group: [k_g, q_{4g}, .., q_{4g+3}]
            def head_list(g):
                return [NQ + g, 4 * g, 4 * g + 1, 4 * g + 2, 4 * g + 3]

            wqk_t: dict = {}

            def fetch_w(h, half):
                wqk_t[(h, half)] = wqk_p.tile(
                    [128, 16, 128], bf16, tag="wqk", name=f"wqk{h}_{half}")
                nc.sync.dma_start(
                    out=wqk_t[(h, half)][:], in_=wqk_d[h, :, ts(half, 16), :])

            fetch_w(head_list(0)[0], 0)
            for g in range(4):
                heads = head_list(g)
                for j, h in enumerate(heads):
                    # prefetch: this head's second half + next head's first
                    fetch_w(h, 1)
                    nxt = heads[j + 1] if j + 1 < 5 else (
                        head_list(g + 1)[0] if g + 1 < 4 else None)
                    if nxt is not None:
                        fetch_w(nxt, 0)
                    ps = psQK.tile([128, T], f32, tag="psQK")
                    wt0 = wqk_t.pop((h, 0))
                    wt1 = wqk_t.pop((h, 1))

                    def mk_cc(cc, wt, base):
                        def step():
                            nc.tensor.matmul(
                                ps[:, 0:512], wt[:, cc - base, :],
                                xT_sb[:, cc, 0:512],
                                start=(cc == 0), stop=(cc == NCC - 1),
                            )
                            nc.tensor.matmul(
                                ps[:, 512:1024], wt[:, cc - base, :],
                                xT_sb[:, cc, 512:1024],
                                start=(cc == 0), stop=(cc == NCC - 1),
                            )
                        return step

                    qk_steps = [mk_cc(cc, wt0, 0) for cc in range(16)]
                    qk_steps += [mk_cc(cc, wt1, 16) for cc in range(16, NCC)]
                    inserts = []
                    if g >= 1 and j >= 1:
                        inserts = (scores_steps(g - 1, j - 1)
                                   + avdn_steps(g - 1, j - 1))
                    interleave(qk_steps, inserts)
                    # RoPE: out = ps*csg + swap_halves(ps)*ssg
                    sw = sw_p.tile([128, T], bf16, tag="sw")
                    nc.scalar.copy(out=sw[0:64, :], in_=ps[64:128, :])
                    nc.scalar.copy(out=sw[64:128, :], in_=ps[0:64, :])
                    m1 = m1_p.tile([128, T], bf16, tag="m1")
                    nc.vector.tensor_mul(m1[:], ps[:], csg[:])
                    m2 = m2_p.tile([128, T], bf16, tag="m2")
                    nc.gpsimd.tensor_mul(m2[:], sw[:], ssg[:])
                    qt = qkT_p.tile([128, T], bf16, tag="qkT", name=f"qkT{h}")
                    nc.vector.tensor_add(qt[:], m1[:], m2[:])
                    qkT[h] = qt

        # ================= PHASE C: last-group attention + proj ===========
        with (
            tc.tile_pool(name="wpp", bufs=4) as wp_p,
            tc.tile_pool(name="ostage", bufs=3) as ostage_p,
        ):
            wp_t: dict = {}

            def fetch_wp(ccol):
                lo = wp_p.tile([128, 8, 512], bf16, tag="wp", name=f"wpl{ccol}")
                hi = wp_p.tile([128, 8, 512], bf16, tag="wp", name=f"wph{ccol}")
                nc.sync.dma_start(out=lo[:], in_=wp_d[ccol, :, 0:8, :])
                nc.sync.dma_start(out=hi[:], in_=wp_d[ccol, :, 8:16, :])
                wp_t[ccol] = (lo, hi)

            fetch_wp(0)
            # last group's attention, software-pipelined one head ahead
            for s in scores_steps(3, 0):
                s()
            for r in range(3):
                interleave(scores_steps(3, r + 1), avdn_steps(3, r))
            for s in avdn_steps(3, 3):
                s()

            for ccol in range(8):
                if ccol + 1 < 8:
                    fetch_wp(ccol + 1)
                lo, hi = wp_t.pop(ccol)
                for tt in range(NTT):
                    po = psQK.tile([128, 512], f32, tag="psQK")
                    for ycc in range(16):
                        wtile = lo if ycc < 8 else hi
                        nc.tensor.matmul(
                            po[:],
                            yts[ycc][:, ts(tt, 128)],
                            wtile[:, ycc % 8, :],
                            start=(ycc == 0),
                            stop=(ycc == 15),
                        )
                    ot = ostage_p.tile([128, 512], f32, tag="os")
                    nc.scalar.copy(out=ot[:], in_=po[:])
                    nc.sync.dma_start(
                        out=out_d[ts(tt, 128), ts(ccol, 512)], in_=ot[:]
                    )

    nc.compile()
    return nc


def prep_inputs(x, Wqkv, Wproj, freqs_cos, freqs_sin):
    """Build the 8 per-core input maps (host-side shard + layout prep)."""
    x = np.asarray(x, np.float32)
    Wqkv = np.asarray(Wqkv, np.float32)
    Wproj = np.asarray(Wproj, np.float32)
    cos = np.asarray(freqs_cos, np.float32)
    sin = np.asarray(freqs_sin, np.float32)

    perm = np.concatenate([np.arange(0, HD, 2), np.arange(1, HD, 2)])
    csg = np.ascontiguousarray(
        np.vstack([cos.T, cos.T]).astype(BF16))            # [128, T]
    ssg = np.ascontiguousarray(
        np.vstack([-sin.T, sin.T]).astype(BF16))           # [128, T]
    # mask[p, i, f] = 1.0 if 128*i + p <= f else 0 (diagonal 256-chunk pair)
    mask = (
        (128 * np.arange(2)[None, :, None] + np.arange(128)[:, None, None])
        <= np.arange(256)[None, None, :]
    ).astype(BF16)
    mask = np.ascontiguousarray(mask)

    in_maps = []
    for c in range(8):
        b, hh = divmod(c, 2)
        qcols = (hh * NQ * HD + (np.arange(NQ) * HD)[:, None] + perm[None, :]).ravel()
        kcols = (
            H * HD + hh * NKV * HD + (np.arange(NKV) * HD)[:, None] + perm[None, :]
        ).ravel()
        vcols = (
            (H + KV) * HD
            + hh * NKV * HD
            + (np.arange(NKV) * HD)[:, None]
            + np.arange(HD)[None, :]
        ).ravel()
        Wqk = Wqkv[:, np.concatenate([qcols, kcols])]      # [4096, 2560]
        # [h, p, cc, col]
        wqk = np.ascontiguousarray(
            Wqk.reshape(NCC, 128, QK_HEADS, 128).transpose(2, 1, 0, 3)
            .astype(BF16))
        Wv = Wqkv[:, vcols]                                # [4096, 512]
        wv = np.ascontiguousarray(
            Wv.reshape(NCC, 128, NKV * HD).transpose(1, 0, 2).astype(BF16))
        Wp = Wproj[hh * NQ * HD : (hh + 1) * NQ * HD, :]   # [2048, 4096]
        wp = np.ascontiguousarray(
            Wp.reshape(16, 128, 8, 512).transpose(2, 1, 0, 3).astype(BF16))
        xT = np.ascontiguousarray(x[b].T.astype(BF16))     # [4096, 1024]
        in_maps.append(
            {"xT": xT, "wqk": wqk, "wv": wv, "wp": wp,
             "csg": csg, "ssg": ssg, "maskd": mask}
        )
    return in_maps


def _get_nc():
    if "nc" not in _CACHE:
        _CACHE["nc"] = _build_nc()
    return _CACHE["nc"]


def kernel(x, Wqkv, Wproj, freqs_cos, freqs_sin, mask=None):
    from concourse.bass_utils import run_bass_kernel_spmd

    nc = _get_nc()
    in_maps = prep_inputs(x, Wqkv, Wproj, freqs_cos, freqs_sin)
    res = run_bass_kernel_spmd(nc, in_maps, core_ids=list(range(8)))
    outs = [res.results[c]["out"] for c in range(8)]
    y = np.stack([outs[2 * b] + outs[2 * b + 1] for b in range(B)], axis=0)
    return y.astype(np.float32)
